# revision 1
# baseline (speedup 1.0000x reference)
"""Trainium2 kernel for nn_AttentionModel_PCA (embedding_lookup).

Math: with sf = softmax(Q^T K) per head,
  G[i,m,a] = sum_h sum_j sf[h,i,j] * V[h,a,Z2[j,m]]
           = sum_{(j,c)} T[(j,c),(i,a)] * E[(j,c),m]
where T[(j,c),(i,a)] = sum_h sf[h,i,j] V[h,a,c]  (tiny H=8 contraction)
and E is the one-hot expansion of Z2. The (5376 x 5376) @ (5376 x M)
GEMM producing G is the dominant cost and runs on the 8 NeuronCores
with M sharded (512 samples per core). Host does the cheap prep
(softmax, T, E) and the small tail (take_along_axis, logsumexp,
weighted sums, regularizer).

Device GEMM runs in fp8e4m3 with DoubleRow perf mode (2 fp8 weights
per PE cell -> 256-deep contraction per pass) or bf16. fp32 matmul
streams at 1/4 rate on TRN2, so fp8-DR is ~6x PE throughput vs the
fp32 version; final-scalar error from fp8 T quantization is ~5e-4
(the one-hot E is exact, PSUM accumulates fp32, and the logsumexp
tail averages out per-element noise).
"""

import sys

import numpy as np
import ml_dtypes

for _p in ("/opt/trn_rl_repo",):
    if _p not in sys.path:
        sys.path.append(_p)

H, d, N1, N2, q1, q2, M = 8, 64, 256, 256, 21, 21, 4096
NCORES = 8
MS = M // NCORES          # 512 samples per core
KDIM = N2 * q2            # 5376 contraction (j,c)
PDIM = N1 * q1            # 5376 output rows (i,a)
KT = KDIM // 128          # 42 contraction tiles (bf16 mode)
KT2 = KDIM // 256         # 21 double-row contraction tiles (fp8 mode)
PT = PDIM // 128          # 42 output-row tiles
NS = 6                    # T-slab ring slots
NB = 8                    # PSUM banks ping-ponged
NOT = 6                   # output ring slots
LAMBD = 0.001

MODE = "fp8dr"            # "fp8dr" | "bf16"
FP8_SCALE = 8.0           # |T|max ~16 -> scaled max ~128 < 240 (e4m3 legacy max)

_PROGRAMS = {}


def _build_program(mode):
    """Raw bass pipeline: explicit standalone wait_ge + then_inc.

    sync  : E load, then 42 per-p-tile T slab loads (NS-slot ring)
    tensor: 42 p-tiles x (21 double-row | 42 plain) accumulating matmuls,
            NB PSUM banks round-robin
    vector: PSUM -> SBUF copies (NOT-slot out ring, gated by stores)
    scalar: SBUF -> G stores (HWDGE)
    """
    import concourse.bass as bass
    import concourse.mybir as mybir

    nc = bass.Bass()
    f32 = mybir.dt.float32
    if mode == "fp8dr":
        mdt = mybir.dt.float8e4
        kt, ko = KT2, 2
        perf = mybir.MatmulPerfMode.DoubleRow
    else:
        mdt = mybir.dt.bfloat16
        kt, ko = KT, 1
        perf = None
    # host-preswizzled layouts (partition dim first, fully contiguous):
    #   Tt[p][ki][t][o][pp] : lhsT tile for (p, t) = [128, (o), 128]
    #   E [ki][t][o][m]     : rhs tile for t = [128, (o), MS]
    Tt = nc.declare_dram_parameter("Tt", [PT, 128, kt * ko * 128], mdt,
                                   isOutput=False)
    E = nc.declare_dram_parameter("E", [128, kt * ko * MS], mdt, isOutput=False)
    G = nc.declare_dram_parameter("G", [PDIM, MS], f32, isOutput=True)

    from contextlib import ExitStack

    ec = 3 if kt % 3 == 0 else 6          # E supertiles per chunk
    nec = kt // ec                        # number of E chunks

    with ExitStack() as stack:
        ent = stack.enter_context
        E_sb = ent(nc.sbuf_tensor([128, kt, ko, MS], mdt))
        slab = ent(nc.sbuf_tensor([128, NS, kt, ko, 128], mdt))
        ot = ent(nc.sbuf_tensor([128, NOT, MS], f32))
        acc = ent(nc.psum_tensor([128, NB * MS], f32))
        # One semaphore per E chunk / ring slot: a DMA's 16 SDMA engines
        # increment its semaphore independently, so with >1 DMA in flight
        # on one shared counting sem, partial sums from two transfers can
        # fake a full count while neither has fully landed. Per-slot sems
        # make every wait exact.
        e_sems = [ent(nc.semaphore(f"e{c}")) for c in range(nec)]
        sl_sems = [ent(nc.semaphore(f"sl{s}")) for s in range(NS)]
        st_sems = [ent(nc.semaphore(f"st{s}")) for s in range(NOT)]
        pe_cnt = ent(nc.semaphore("pe_cnt"))   # p-tiles fully consumed by PE
        cp_sem = ent(nc.semaphore("cp_sem"))   # PSUM->SBUF copies done
        block = ent(nc.Block())

        @block.sync
        def _(sync):
            # Head: the DMA ring drains roughly FIFO at line rate, so issue
            # in the order the PE needs data: slab0, then E chunks (p-tile
            # 0 streams right behind them), with slab1 slotted mid-E so it
            # lands about when p-tile 0 drains.
            sync.dma_start(slab[:, 0], Tt[0, :, :]).then_inc(sl_sems[0], 16)
            for c in range(nec):
                sync.dma_start(
                    E_sb[:, c * ec:(c + 1) * ec],
                    E[:, c * ec * ko * MS:(c + 1) * ec * ko * MS],
                ).then_inc(e_sems[c], 16)
                if c == 2:
                    sync.dma_start(slab[:, 1], Tt[1, :, :]
                                   ).then_inc(sl_sems[1], 16)
            # hold further slab prefetch until E has fully landed so it
            # cannot steal DMA bandwidth from (or reorder around) the load
            # every p-tile depends on
            for c in range(nec):
                sync.wait_ge(e_sems[c], 16)
            for p in range(2, PT):
                if p >= NS:
                    sync.wait_ge(pe_cnt, p - NS + 1)
                sync.dma_start(slab[:, p % NS], Tt[p, :, :]
                               ).then_inc(sl_sems[p % NS], 16)

        def one_mm(p, t):
            b = (p % NB) * MS
            if mode == "fp8dr":
                lhsT = slab[:, p % NS, t, :, :]
                rhs = E_sb[:, t, :, :]
            else:
                lhsT = slab[:, p % NS, t, 0, :]
                rhs = E_sb[:, t, 0, :]
            mm = nc.tensor.matmul(
                acc[:, b:b + MS], lhsT, rhs,
                start=(t == 0), stop=(t == kt - 1), perf_mode=perf,
            )
            if t == kt - 1:
                mm.then_inc(pe_cnt, 1)

        @block.tensor
        def _(tensor):
            for p in range(PT):
                tensor.wait_ge(sl_sems[p % NS], 16 * (p // NS + 1))
                if p == 1:
                    # all E landed before unguarded p-tiles (p0 waits
                    # per chunk)
                    for c in range(nec):
                        tensor.wait_ge(e_sems[c], 16)
                if p >= NB:
                    # bank reused from p-NB: wait for its copy-out
                    tensor.wait_ge(cp_sem, p - NB + 1)
                for t in range(kt):
                    if p == 0 and t % ec == 0:
                        tensor.wait_ge(e_sems[t // ec], 16)
                    one_mm(p, t)

        @block.vector
        def _(vector):
            for p in range(PT):
                vector.wait_ge(pe_cnt, p + 1)
                if p >= NOT:
                    # ot slot reused from p-NOT: wait for its store
                    vector.wait_ge(st_sems[p % NOT], 16 * (p // NOT))
                nc.vector.tensor_copy(
                    ot[:, p % NOT, :], acc[:, (p % NB) * MS:(p % NB + 1) * MS],
                ).then_inc(cp_sem, 1)

        @block.scalar
        def _(scalar):
            for p in range(PT):
                scalar.wait_ge(cp_sem, p + 1)
                scalar.dma_start(
                    G[p * 128:(p + 1) * 128, :], ot[:, p % NOT, :]
                ).then_inc(st_sems[p % NOT], 16)

    return nc


def host_prep(Q, K, V, Z2):
    """softmax, T (preswizzled + quantized for the PE), one-hot row ids."""
    e = np.einsum("hdi,hdj->hij", Q, K, optimize=True)
    e -= e.max(axis=2, keepdims=True)
    np.exp(e, out=e)
    sf = e / e.sum(axis=2, keepdims=True)
    Tt = np.einsum("hij,hac->jcia", sf, V, optimize=True).reshape(KDIM, PDIM)
    Tt = np.ascontiguousarray(Tt, np.float32)

    if MODE == "fp8dr":
        # Tb[p, ki, t2, o, pp] = s*T[t2*256 + o*128 + ki, p*128 + pp]
        Tq = (Tt * FP8_SCALE).astype(ml_dtypes.float8_e4m3)
        Tb = np.ascontiguousarray(
            Tq.reshape(KT2, 2, 128, PT, 128).transpose(3, 2, 0, 1, 4)
        ).reshape(PT, 128, KT2 * 2 * 128)
        Tdeq = Tq.astype(np.float32) / FP8_SCALE   # for spot checks
    else:
        Tq = Tt.astype(ml_dtypes.bfloat16)
        Tb = np.ascontiguousarray(
            Tq.reshape(KT, 128, PT, 128).transpose(2, 1, 0, 3)
        ).reshape(PT, 128, KT * 128)
        Tdeq = Tq.astype(np.float32)

    # one-hot row index per (j, m): k = j*q2 + Z2[j,m]
    rows = (np.arange(N2, dtype=np.int64)[:, None] * q2 + Z2.astype(np.int64))
    return sf, Tb, Tdeq, rows


def build_E(rows_c):
    """Per-core one-hot E in the device layout [128, kt*ko*MS]."""
    Mloc = rows_c.shape[1]
    dt = ml_dtypes.float8_e4m3 if MODE == "fp8dr" else ml_dtypes.bfloat16
    Eoh = np.zeros((KDIM, Mloc), dt)
    Eoh[rows_c, np.arange(Mloc, dtype=np.int64)[None, :]] = 1.0
    if MODE == "fp8dr":
        Eb = np.ascontiguousarray(
            Eoh.reshape(KT2, 2, 128, Mloc).transpose(2, 0, 1, 3))
    else:
        Eb = np.ascontiguousarray(Eoh.reshape(KT, 128, Mloc).transpose(1, 0, 2))
    return Eb.reshape(128, -1)


def host_tail(G, sf, V, Z1, weights):
    """take_along_axis + logsumexp + loss + regularizer on (N1, M, q1) G."""
    Z1i = Z1.astype(np.int64)
    mat_ene_sum = np.take_along_axis(G, Z1i[:, :, None], axis=2)[..., 0].sum(axis=0)

    Gm = G.max(axis=0)                                   # (M, q1)
    L = np.log(np.exp(G - Gm).sum(axis=0)) + Gm          # (M, q1)
    mx = np.maximum(L.max(axis=1), 0.0)
    logZ = np.log(np.exp(L - mx[:, None]).sum(axis=1)
                  + (N1 - q1) * np.exp(-mx)) + mx

    pl = -(weights.astype(np.float64)
           * (mat_ene_sum.astype(np.float64) - logZ.astype(np.float64))).sum()

    sf2 = sf.reshape(H, -1).astype(np.float64)
    VV = V.reshape(H, -1).astype(np.float64)
    reg = LAMBD * ((sf2 @ sf2.T) * (VV @ VV.T)).sum()
    return np.array(pl + reg, dtype=np.float32)


def run_device(Tb, rows, trace=False, **kw):
    from concourse.bass_utils import run_bass_kernel_spmd

    if MODE not in _PROGRAMS:
        _PROGRAMS[MODE] = _build_program(MODE)
    in_maps = [
        {"Tt": Tb, "E": build_E(rows[:, c * MS:(c + 1) * MS])}
        for c in range(NCORES)
    ]
    out = run_bass_kernel_spmd(_PROGRAMS[MODE], in_maps, list(range(NCORES)),
                               trace=trace, **kw)
    Gf = np.concatenate([np.asarray(out.results[c]["G"]) for c in range(NCORES)],
                        axis=1)                          # (PDIM, M)
    if MODE == "fp8dr":
        Gf = Gf / FP8_SCALE
    return Gf, out


def kernel(**inputs):
    Q = np.asarray(inputs["Q"], np.float32)
    K = np.asarray(inputs["K"], np.float32)
    V = np.asarray(inputs["V"], np.float32)
    Z1 = np.asarray(inputs["Z1"])
    Z2 = np.asarray(inputs["Z2"])
    weights = np.asarray(inputs["weights"], np.float32)

    sf, Tb, _, rows = host_prep(Q, K, V, Z2)
    Gf, _ = run_device(Tb, rows)
    G = Gf.reshape(N1, q1, M).transpose(0, 2, 1)         # (N1, M, q1)
    return host_tail(G, sf, V, Z1, weights)



# revision 2
# speedup vs baseline: 2.3343x; 2.3343x over previous
"""Trainium2 kernel for nn_AttentionModel_PCA (embedding_lookup).

Math: with sf = softmax(Q^T K) per head,
  G[i,a,m] = sum_h sum_j sf[h,i,j] * V[h,a,Z2[j,m]]

Instead of the one-hot (j,c) blow-up (K=5376 dense GEMM, 118e9 MACs),
contract over (h,j) = 2048:
  G[i,(a,m)] = sum_{(h,j)} sfT[(h,j), i] * Vg[(h,j), (a,m)]
with Vg[(h,j),(a,m)] = V[h,a,Z2[j,m]] gathered on the host (cheap fancy
index from the tiny 21-entry V tables). That is 45e9 MACs total, 2.6x
fewer than the dense-E path, ~172k PE cycles/core in fp8 DoubleRow.

Device per core (M sharded, Ms=512): lhsT = sf tiles (512 KB, fully
resident -> only 16 LDWEIGHTS per a), rhs = Vg streamed in 21 a-slabs
of [128, 8kt, 2ko, 512m] fp8 (1.05 MB each) through an NS-slot ring,
overlapping DMA with the PE. Out G[(a,it), i128, m] in bf16. Host does
softmax/quantize/gather prep and the tail (take_along_axis, logsumexp,
loss, regularizer).

Scales: sf*16 and Vg*8 keep fp8e4m3 in range (max ~16 / ~30 << 240);
PSUM accumulates fp32; the PSUM->SBUF copy multiplies by 1/128.
"""

import sys

import numpy as np
import ml_dtypes

for _p in ("/opt/trn_rl_repo",):
    if _p not in sys.path:
        sys.path.append(_p)

H, d, N1, N2, q1, q2, M = 8, 64, 256, 256, 21, 21, 4096
NCORES = 8
MS = M // NCORES          # 512 samples per core
KT = 8                    # contraction tiles over (h,j): kt == h
KO = 2                    # DoubleRow pair dim (j high bit)
IT = 2                    # i tiles (256 / 128)
NA = q1                   # 21 a iterations
NU = NA * IT              # 42 units (a-major, itile inner)
NS = 6                    # Vg a-slab ring slots
NB = 8                    # PSUM banks ping-ponged
NOT = 6                   # output ring slots
LAMBD = 0.001

SF_SCALE = 16.0
VG_SCALE = 8.0
DEQ = 1.0 / (SF_SCALE * VG_SCALE)

_PROGRAMS = {}


def _build_program():
    """Raw bass pipeline: explicit standalone wait_ge + then_inc.

    sync  : sfW load, then 21 per-a Vg slab loads (NS-slot ring)
    tensor: 21 a x 2 itiles x 8 accumulating DR matmuls, NB PSUM banks
    vector: PSUM -> SBUF dequant copies (bf16, NOT-slot out ring)
    scalar: SBUF -> G stores (HWDGE)
    """
    import concourse.bass as bass
    import concourse.mybir as mybir

    nc = bass.Bass()
    f8 = mybir.dt.float8e4
    bf16 = mybir.dt.bfloat16
    f32 = mybir.dt.float32
    perf = mybir.MatmulPerfMode.DoubleRow

    # host-preswizzled layouts (partition dim first, fully contiguous):
    #   sfW[ki][kt][ko][it][is] : lhsT tile for (kt, it) = [128, 2, 128]
    #   Vg [a][ki][kt][ko][m]   : rhs slab for a = [128, 8*2*512]
    sfW = nc.declare_dram_parameter("sfW", [128, KT * KO * IT * 128], f8,
                                    isOutput=False)
    Vg = nc.declare_dram_parameter("Vg", [NA, 128, KT * KO * MS], f8,
                                   isOutput=False)
    G = nc.declare_dram_parameter("G", [NU, 128, MS], bf16, isOutput=True)

    from contextlib import ExitStack

    with ExitStack() as stack:
        ent = stack.enter_context
        sf_sb = ent(nc.sbuf_tensor([128, KT, KO, IT, 128], f8))
        slab = ent(nc.sbuf_tensor([128, NS, KT, KO, MS], f8))
        ot = ent(nc.sbuf_tensor([128, NOT, MS], bf16))
        acc = ent(nc.psum_tensor([128, NB * MS], f32))
        sf_sem = ent(nc.semaphore("sf_sem"))
        sl_sems = [ent(nc.semaphore(f"sl{s}")) for s in range(NS)]
        st_sems = [ent(nc.semaphore(f"st{s}")) for s in range(NOT)]
        pe_cnt = ent(nc.semaphore("pe_cnt"))   # units fully consumed by PE
        cp_sem = ent(nc.semaphore("cp_sem"))   # PSUM->SBUF copies done
        block = ent(nc.Block())

        @block.sync
        def _(sync):
            sync.dma_start(sf_sb[:, :], sfW[:, :]).then_inc(sf_sem, 16)
            for a in range(NA):
                if a >= NS:
                    # slab slot reused from a-NS: both its units consumed
                    sync.wait_ge(pe_cnt, (a - NS + 1) * IT)
                sync.dma_start(slab[:, a % NS], Vg[a, :, :]
                               ).then_inc(sl_sems[a % NS], 16)

        @block.tensor
        def _(tensor):
            tensor.wait_ge(sf_sem, 16)
            for a in range(NA):
                tensor.wait_ge(sl_sems[a % NS], 16 * (a // NS + 1))
                for it in range(IT):
                    u = a * IT + it
                    b = (u % NB) * MS
                    if u >= NB:
                        tensor.wait_ge(cp_sem, u - NB + 1)
                    for kt in range(KT):
                        mm = nc.tensor.matmul(
                            acc[:, b:b + MS],
                            sf_sb[:, kt, :, it, :],
                            slab[:, a % NS, kt, :, :],
                            start=(kt == 0), stop=(kt == KT - 1),
                            perf_mode=perf,
                        )
                        if kt == KT - 1:
                            mm.then_inc(pe_cnt, 1)

        @block.vector
        def _(vector):
            for u in range(NU):
                vector.wait_ge(pe_cnt, u + 1)
                if u >= NOT:
                    vector.wait_ge(st_sems[u % NOT], 16 * (u // NOT))
                nc.vector.tensor_scalar_mul(
                    ot[:, u % NOT, :], acc[:, (u % NB) * MS:(u % NB + 1) * MS],
                    DEQ,
                ).then_inc(cp_sem, 1)

        @block.scalar
        def _(scalar):
            for u in range(NU):
                scalar.wait_ge(cp_sem, u + 1)
                scalar.dma_start(
                    G[u, :, :], ot[:, u % NOT, :]
                ).then_inc(st_sems[u % NOT], 16)

    return nc


def host_prep(Q, K, V, Z2):
    """softmax, quantized+preswizzled sf weights and per-core Vg slabs."""
    e = np.einsum("hdi,hdj->hij", Q, K, optimize=True)
    e -= e.max(axis=2, keepdims=True)
    np.exp(e, out=e)
    sf = e / e.sum(axis=2, keepdims=True)

    # sfW[ki][kt=h][ko][it][is] = (sf*16)[h, it*128+is, ko*128+ki]
    sf8 = (sf * SF_SCALE).astype(ml_dtypes.float8_e4m3)
    sfW = np.ascontiguousarray(
        sf8.reshape(H, IT, 128, KO, 128).transpose(4, 0, 3, 1, 2)
    ).reshape(128, KT * KO * IT * 128)

    # Vg[a][ki][kt=h][ko][m] = (V*8)[h, a, Z2[ko*128+ki, m]]
    V8 = (V * VG_SCALE).astype(ml_dtypes.float8_e4m3)
    Vgf = V8[:, :, Z2.astype(np.int64)]            # (H, q1, N2, M)
    return sf, sf8, V8, sfW, Vgf


def build_vg(Vgf_c):
    """Per-core Vg slab tensor [NA, 128, KT*KO*MS] from (H, q1, N2, Ms)."""
    Mloc = Vgf_c.shape[3]
    return np.ascontiguousarray(
        Vgf_c.reshape(H, NA, KO, 128, Mloc).transpose(1, 3, 0, 2, 4)
    ).reshape(NA, 128, KT * KO * Mloc)


def host_tail(G, sf, V, Z1, weights):
    """take_along_axis + logsumexp + loss + regularizer on (N1, M, q1) G."""
    Z1i = Z1.astype(np.int64)
    mat_ene_sum = np.take_along_axis(G, Z1i[:, :, None], axis=2)[..., 0].sum(axis=0)

    Gm = G.max(axis=0)                                   # (M, q1)
    L = np.log(np.exp(G - Gm).sum(axis=0)) + Gm          # (M, q1)
    mx = np.maximum(L.max(axis=1), 0.0)
    logZ = np.log(np.exp(L - mx[:, None]).sum(axis=1)
                  + (N1 - q1) * np.exp(-mx)) + mx

    pl = -(weights.astype(np.float64)
           * (mat_ene_sum.astype(np.float64) - logZ.astype(np.float64))).sum()

    sf2 = sf.reshape(H, -1).astype(np.float64)
    VV = V.reshape(H, -1).astype(np.float64)
    reg = LAMBD * ((sf2 @ sf2.T) * (VV @ VV.T)).sum()
    return np.array(pl + reg, dtype=np.float32)


def run_device(sfW, Vgf, trace=False, **kw):
    from concourse.bass_utils import run_bass_kernel_spmd

    if "prog" not in _PROGRAMS:
        _PROGRAMS["prog"] = _build_program()
    in_maps = [
        {"sfW": sfW, "Vg": build_vg(Vgf[:, :, :, c * MS:(c + 1) * MS])}
        for c in range(NCORES)
    ]
    out = run_bass_kernel_spmd(_PROGRAMS["prog"], in_maps, list(range(NCORES)),
                               trace=trace, **kw)
    # G[u= a*2+it][i128][m] -> (N1, q1, Mloc) -> concat m
    Gf = np.concatenate(
        [np.asarray(out.results[c]["G"]).astype(np.float32)
         .reshape(NA, IT, 128, MS).transpose(1, 2, 0, 3).reshape(N1, NA, MS)
         for c in range(NCORES)],
        axis=2)                                          # (N1, q1, M)
    return Gf, out


def kernel(**inputs):
    Q = np.asarray(inputs["Q"], np.float32)
    K = np.asarray(inputs["K"], np.float32)
    V = np.asarray(inputs["V"], np.float32)
    Z1 = np.asarray(inputs["Z1"])
    Z2 = np.asarray(inputs["Z2"])
    weights = np.asarray(inputs["weights"], np.float32)

    sf, _, _, sfW, Vgf = host_prep(Q, K, V, Z2)
    Gf, _ = run_device(sfW, Vgf)
    G = Gf.transpose(0, 2, 1)                            # (N1, M, q1)
    return host_tail(G, sf, V, Z1, weights)
